# revision 8
# baseline (speedup 1.0000x reference)
"""Trainium2 Bass kernel for nn_Attention_86672440033867 (relative-position attention).

Sharding: head-parallel over 8 NeuronCores (1 head per core, all 16 batches).
Each core computes, for its head h:
  qkvT = w_qkv_h^T @ x^T           (col-tiled matmuls, 4 batches concurrently)
  S^T  = k_b q_b^T                 (row-tiled matmuls, 4 batches concurrently)
  P^T  = exp(SCALE*S^T) * exp(B)^T (ACT exp + DVE/GPSIMD multiply; bias via
                                    host-gathered exp(bias) table, batch-invariant)
  O^T  = v_b^T P^T (+ ones row -> softmax denominators)   (col-tiled)
  out_partial = (O^T / denom)^T @ w_out_h                  (row-tiled + normalize)
Host sums the 8 partial projections and adds b_out.

The relative_index gather is resolved on the host: bias = table[relative_index]
is batch-independent, so exp(bias^T) is computed once per head and kept
resident in SBUF (2 MB bf16), amortized over all 16 batches.
"""
import numpy as np
import ml_dtypes
from contextlib import ExitStack

import concourse.bass as bass
import concourse.mybir as mybir
import concourse.tile as tile
from concourse import bacc
from concourse.bass_utils import run_bass_kernel_spmd

BF16 = mybir.dt.bfloat16
F32 = mybir.dt.float32

HEADS = 8
D = 32          # head dim
INP = 384
OUP = 384
SCALE = D ** -0.5
AF = mybir.ActivationFunctionType


def build_kernel(NB=16, N=1024, num_devices=8, loop_k=0):
    """Build the per-core Bass module. NB = total batches, N = tokens/batch."""
    assert NB % 4 == 0 and N % 128 == 0
    NG = NB // 4            # groups of 4 batches
    NJC = N // 128          # key chunks (128) per batch
    IH = min(512, N)        # query-column tile width
    NIH = N // IH           # query halves per batch
    NTC = IH // 128         # token chunks (128) per query tile
    TOK = NB * N

    nc = bacc.Bacc("TRN2", target_bir_lowering=False, num_devices=num_devices)

    xt_d = nc.dram_tensor("xt", [INP, TOK], BF16, kind="ExternalInput")
    wqkv_d = nc.dram_tensor("wqkv", [3, 128, 96], BF16, kind="ExternalInput")
    wout4_d = nc.dram_tensor("wout4", [128, OUP], BF16, kind="ExternalInput")
    expb_d = nc.dram_tensor("expb", [128, NJC, N], BF16, kind="ExternalInput")
    ident_d = nc.dram_tensor("ident", [128, 32], BF16, kind="ExternalInput")
    outp_d = nc.dram_tensor("outp", [TOK, OUP], BF16, kind="ExternalOutput")

    with tile.TileContext(nc) as tc, ExitStack() as ctx:
        const = ctx.enter_context(tc.tile_pool(name="const", bufs=1))
        big = ctx.enter_context(tc.tile_pool(name="big", bufs=1))

        wqkv_sb = const.tile([128, 3, 96], BF16)
        wout_sb = const.tile([128, OUP], BF16)
        ident_sb = const.tile([128, 32], BF16)
        ones_sb = const.tile([128, 32], BF16)
        expb_sb = const.tile([128, NJC, N], BF16)
        for kc in range(3):
            nc.sync.dma_start(wqkv_sb[:, kc, :], wqkv_d.ap()[kc])
        nc.sync.dma_start(wout_sb[:], wout4_d.ap())
        nc.sync.dma_start(ident_sb[:], ident_d.ap())
        nc.sync.dma_start(expb_sb[:], expb_d.ap())
        nc.vector.memset(ones_sb[:], 1.0)

        # Resident activation layouts
        QK = big.tile([128, NG * 2 * N], BF16)       # per group: [q(N) | k(N)], batch r on partitions 32r
        V_sb = big.tile([128, NB * NJC * 33], BF16)  # v natural [j,d] per (b,jc) + ones col
        OT0 = big.tile([128, NG * N], BF16)          # attn out^T pairs (b%4 in {0,1}): bA rows 0:33, bB rows 64:97
        OT1 = big.tile([128, NG * N], BF16)          # same for (b%4 in {2,3})
        den_nat = big.tile([128, NB * NJC], BF16)    # denominators, natural layout [tok128, chunk]
        recip_nat = big.tile([128, NB * NJC], F32)

        nc.gpsimd.memset(V_sb[:], 1.0)  # ones column pre-fill; v blocks overwritten below

        xt_pool = ctx.enter_context(tc.tile_pool(name="xt", bufs=14))
        v4t_pool = ctx.enter_context(tc.tile_pool(name="v4t", bufs=2))
        es_pool = ctx.enter_context(tc.tile_pool(name="es", bufs=3))
        pt_pool = ctx.enter_context(tc.tile_pool(name="pt", bufs=NJC + 2))
        out_pool = ctx.enter_context(tc.tile_pool(name="outp", bufs=6))

        # ---------------- Stage A: qkv projections + v transposes ----------------
        from contextlib import nullcontext
        loopA = tc.For_i(0, loop_k, 1) if loop_k else nullcontext()
        with tc.tile_pool(name="ps_qkv", bufs=4, space="PSUM") as ps_qkv, \
             tc.tile_pool(name="ps_vt", bufs=1, space="PSUM") as ps_vt, loopA:
            for g in range(NG):
                v4t = v4t_pool.tile([128, N], BF16, tag="v4t")
                for ih in range(NIH):
                    xts = {}
                    for kc in range(3):
                        for c in range(4):
                            t = xt_pool.tile([128, IH], BF16, tag="xt")
                            nc.sync.dma_start(
                                t[:], xt_d.ap()[kc * 128:(kc + 1) * 128,
                                                (4 * g + c) * N + ih * IH:
                                                (4 * g + c) * N + (ih + 1) * IH])
                            xts[(kc, c)] = t
                    for oi, colbase in enumerate((0, 32, 64)):  # q, k, v
                        ps = ps_qkv.tile([128, IH], F32, tag="ps_qkv")
                        for kc in range(3):
                            for c in range(4):
                                nc.tensor.matmul(
                                    ps[32 * c:32 * (c + 1), :],
                                    wqkv_sb[:, kc, colbase:colbase + 32],
                                    xts[(kc, c)][:],
                                    start=(kc == 0), stop=(kc == 2),
                                    tile_position=(0, 32 * c),
                                    skip_group_check=True)
                        if oi == 0:
                            dst = QK[:, g * 2 * N + ih * IH:g * 2 * N + (ih + 1) * IH]
                        elif oi == 1:
                            dst = QK[:, g * 2 * N + N + ih * IH:g * 2 * N + N + (ih + 1) * IH]
                        else:
                            dst = v4t[:, ih * IH:(ih + 1) * IH]
                        nc.vector.tensor_copy(dst, ps[:])
                # v transposes: vT [32,128] blocks -> v natural [128,32] per batch
                for jc in range(NJC):
                    vt = ps_vt.tile([128, 4, 1024], BF16, tag="ps_vt")
                    for r in range(4):
                        nc.tensor.transpose(
                            vt[:, r, 0:32],
                            v4t[32 * r:32 * (r + 1), jc * 128:(jc + 1) * 128],
                            ident_sb[32 * r:32 * (r + 1), 0:32],
                            tile_position=(32 * r, 0))
                    vv = V_sb[:].rearrange("p (b j e) -> p b j e", j=NJC, e=33)
                    nc.vector.tensor_copy(
                        vv[:, 4 * g:4 * g + 4, jc, 0:32],
                        vt[:, 0:4, 0:32])

        # ---------------- Stage B: attention + output projection ----------------
        loopB = tc.For_i(0, loop_k, 1) if loop_k else nullcontext()
        with tc.tile_pool(name="ps_dots", bufs=1, space="PSUM") as ps_dots, \
             tc.tile_pool(name="ps_av", bufs=2, space="PSUM") as ps_av, \
             tc.tile_pool(name="ps_out", bufs=2, space="PSUM") as ps_out, loopB:
            for g in range(NG):
                for ih in range(NIH):
                    seg = (g * NIH + ih) * IH  # free offset in OT0/OT1
                    pts = []
                    for jc in range(NJC):
                        ps = ps_dots.tile([128, 4 * IH], F32, tag="ps_dots")
                        for r in range(4):
                            nc.tensor.matmul(
                                ps[:, r * IH:(r + 1) * IH],
                                QK[32 * r:32 * (r + 1),
                                   g * 2 * N + N + jc * 128:g * 2 * N + N + (jc + 1) * 128],
                                QK[32 * r:32 * (r + 1),
                                   g * 2 * N + ih * IH:g * 2 * N + (ih + 1) * IH],
                                start=True, stop=True, tile_position=(32 * r, 0))
                        es = es_pool.tile([128, 4 * IH], BF16, tag="es")
                        nc.scalar.activation(es[:], ps[:], AF.Exp, scale=float(SCALE))
                        pt = pt_pool.tile([128, 4 * IH], BF16, tag="pt")
                        for c in range(4):
                            eng = nc.vector if (c + jc) % 2 == 0 else nc.gpsimd
                            eng.tensor_mul(
                                pt[:, c * IH:(c + 1) * IH],
                                es[:, c * IH:(c + 1) * IH],
                                expb_sb[:, jc, ih * IH:(ih + 1) * IH])
                        pts.append(pt)
                    # AV: two batch pairs, col-tiled (v at cols 0/64, ones at 32/96)
                    for p in range(2):
                        av = ps_av.tile([128, IH], F32, tag="ps_av")
                        vv = V_sb[:].rearrange("p (b j e) -> p b j e", j=NJC, e=33)
                        bA = 4 * g + 2 * p
                        bB = bA + 1
                        for jc in range(NJC):
                            pA = pts[jc][:, (2 * p) * IH:(2 * p + 1) * IH]
                            pB = pts[jc][:, (2 * p + 1) * IH:(2 * p + 2) * IH]
                            st = dict(start=(jc == 0), stop=(jc == NJC - 1),
                                      skip_group_check=True)
                            nc.tensor.matmul(av[0:32, :], vv[:, bA, jc, 0:32], pA,
                                             tile_position=(0, 0), **st)
                            nc.tensor.matmul(av[32:64, :], ones_sb[:], pA,
                                             tile_position=(0, 32), **st)
                            nc.tensor.matmul(av[64:96, :], vv[:, bB, jc, 0:32], pB,
                                             tile_position=(0, 64), **st)
                            nc.tensor.matmul(av[96:128, :], ones_sb[:], pB,
                                             tile_position=(0, 96), **st)
                        OTx = OT0 if p == 0 else OT1
                        nc.vector.tensor_copy(OTx[:, seg:seg + IH], av[:])
                    # denominators -> natural layout, reciprocal
                    for r2 in range(4):
                        b = 4 * g + r2
                        OTx = OT0 if r2 < 2 else OT1
                        row = 32 if r2 % 2 == 0 else 96
                        for tcl in range(NTC):
                            nc.sync.dma_start(
                                den_nat[:, b * NJC + ih * NTC + tcl:
                                        b * NJC + ih * NTC + tcl + 1],
                                OTx[row:row + 1,
                                    seg + tcl * 128:seg + (tcl + 1) * 128])
                    dview = den_nat[:].rearrange("p (b e) -> p b e", e=NJC)
                    rview = recip_nat[:].rearrange("p (b e) -> p b e", e=NJC)
                    nc.vector.reciprocal(
                        rview[:, 4 * g:4 * g + 4, ih * NTC:(ih + 1) * NTC],
                        dview[:, 4 * g:4 * g + 4, ih * NTC:(ih + 1) * NTC])
                    # output projection + normalize
                    for r2 in range(4):
                        b = 4 * g + r2
                        OTx = OT0 if r2 < 2 else OT1
                        rbase = 0 if r2 % 2 == 0 else 64
                        for tcl in range(NTC):
                            po = ps_out.tile([128, 512], F32, tag="ps_out")
                            nc.tensor.matmul(
                                po[:, 0:OUP],
                                OTx[rbase:rbase + 32, seg + tcl * 128:seg + (tcl + 1) * 128],
                                wout_sb[rbase:rbase + 32, :],
                                start=True, stop=True, tile_position=(rbase, 0))
                            ot = out_pool.tile([128, OUP], BF16, tag="outp")
                            rc = recip_nat[:, b * NJC + ih * NTC + tcl:
                                           b * NJC + ih * NTC + tcl + 1]
                            if (r2 * NTC + tcl) % 5 == 4:
                                nc.scalar.activation(ot[:], po[:, 0:OUP], AF.Copy,
                                                     scale=rc)
                            else:
                                nc.vector.tensor_scalar_mul(ot[:], po[:, 0:OUP], rc)
                            nc.sync.dma_start(
                                outp_d.ap()[b * N + ih * IH + tcl * 128:
                                            b * N + ih * IH + (tcl + 1) * 128, :],
                                ot[:])
    nc.compile()
    return nc


def _make_relative_index(ih, iw):
    coords = np.stack(np.meshgrid(np.arange(ih), np.arange(iw), indexing='ij'))
    cf = coords.reshape(2, -1)
    rel = cf[:, :, None] - cf[:, None, :]
    rel = rel.transpose(1, 2, 0).astype(np.int64)
    rel[:, :, 0] += ih - 1
    rel[:, :, 1] += iw - 1
    rel[:, :, 0] *= 2 * iw - 1
    return rel.sum(-1)


def host_prep(x, w_qkv, relative_bias_table, relative_index, w_out, NB, N):
    """Build per-core input maps."""
    bf = ml_dtypes.bfloat16
    TOK = NB * N
    NJC = N // 128
    xt = np.ascontiguousarray(x.reshape(TOK, INP).T).astype(bf)
    ident = np.tile(np.eye(32, dtype=np.float32), (4, 1)).astype(bf)
    bias_full = relative_bias_table[relative_index]  # [N, N, H]
    in_maps = []
    for h in range(HEADS):
        w96 = np.concatenate(
            [w_qkv[:, h * D:(h + 1) * D],
             w_qkv[:, 256 + h * D:256 + (h + 1) * D],
             w_qkv[:, 512 + h * D:512 + (h + 1) * D]], axis=1)  # [384, 96]
        wqkv3 = np.ascontiguousarray(w96.reshape(3, 128, 96)).astype(bf)
        wout4 = np.tile(w_out[h * D:(h + 1) * D, :], (4, 1)).astype(bf)
        expbT = np.exp(bias_full[:, :, h].T)  # [j, i]
        expb = np.ascontiguousarray(
            expbT.reshape(NJC, 128, N).transpose(1, 0, 2)).astype(bf)
        in_maps.append({
            "xt": xt, "wqkv": wqkv3, "wout4": wout4,
            "expb": expb, "ident": ident,
        })
    return in_maps


_NC_CACHE = {}


def kernel(x, w_qkv, relative_bias_table, w_out, b_out, relative_index):
    x = np.asarray(x, dtype=np.float32)
    w_qkv = np.asarray(w_qkv, dtype=np.float32)
    relative_bias_table = np.asarray(relative_bias_table, dtype=np.float32)
    w_out = np.asarray(w_out, dtype=np.float32)
    b_out = np.asarray(b_out, dtype=np.float32)
    relative_index = np.asarray(relative_index)

    NB, N, _ = x.shape
    key = (NB, N)
    if key not in _NC_CACHE:
        _NC_CACHE[key] = build_kernel(NB=NB, N=N, num_devices=HEADS)
    nc = _NC_CACHE[key]

    in_maps = host_prep(x, w_qkv, relative_bias_table, relative_index, w_out, NB, N)
    res = run_bass_kernel_spmd(nc, in_maps, core_ids=list(range(HEADS)))
    out = np.zeros((NB * N, OUP), np.float32)
    for r in res.results:
        out += r["outp"].astype(np.float32)
    out += b_out[None, :]
    return out.reshape(NB, N, OUP)
